# revision 58
# baseline (speedup 1.0000x reference)
"""Trainium2 Bass kernel for a spatial self-attention block (fp8 DoubleRow).

reference computation (B=4, H=W=64, C=512, N=H*W=4096):
    h = group_norm(x, gamma, beta, 32 groups)
    q,k,v = h@wq+bq, h@wk+bk, h@wv+bv
    scores = (q @ k^T) / sqrt(C); attn = softmax(scores, -1)
    out = (attn @ v) @ wo + bo + x

Sharding: 8 cores = (batch b in 0..3) x (query-half in 0..1). Each core
computes group-norm stats + K/V for its full batch element (duplicated
across the pair) and attention outputs for its own 2048 query rows.
The host permutes each core's batch rows so its own queries are rows
0:2048 — attention is permutation-invariant over keys, so one uniform
SPMD program works for all cores.

Precision strategy (rel-err budget 2e-2; measured 7.9e-3 on hardware):
  - x arrives channel-major (host-transposed) pre-cast to fp8 e4m3.
  - group-norm stats from the fp8 x: channel sums on DVE reduce_sum,
    channel sum-of-squares on ScalarE (Square + accum_out); per-column
    PE transposes move the [128,8] stat block into rows for the group
    reduction chain (single-partition PSUM reads must start at
    partition 0, hence one [128,1] transpose per stat column).
  - group norm is folded into the projections: h = x*s + t, so
    q = x @ (diag(s) wq) + (t@wq + bq). Weights arrive fp16 scaled by
    16 (keeps the fp8 quantization of s*w in the normal range); the
    fold multiplies by the per-channel s and casts to fp8. The 1/16
    is applied when the projection PSUM is written out.
  - all the big matmuls (QKV projections, scores, attn@V) run fp8
    e4m3 with perf_mode=DoubleRow: 3-D APs [128, 2, m] contract 256
    channels/keys per instruction at 2 rows/cycle.
  - softmax: exp(s/sqrt(C) - 3) computed on ScalarE straight into fp8
    (max score ~6.7 so exp <= 42, inside e4m3's 240 max normal). The
    denominator sums the same fp8 values, so softmax stays exactly
    normalized; V's bias is folded in as bv (x) denom added to the
    unnormalized accumulator.
  - the output projection runs fp16 (the unnormalized attn@V values
    can exceed fp8 range); 1/denominator is applied after it, where
    the query index is the partition dim. bo + x arrive pre-summed
    from the host (xbo).

Packed host constants tensor `consts` [128, 900] (fp32 bits):
  cols 0:128   identity matrix (PE transposes)
  col  128     ones column  [128,1]
  cols 129:257 ones row     [1,128] (partition 0)
"""

import sys

import numpy as np
import ml_dtypes

if "/opt/trn_rl_repo" not in sys.path:
    sys.path.insert(0, "/opt/trn_rl_repo")

import concourse.mybir as mybir
import concourse.tile as tile
from concourse import bacc
from concourse.bass_utils import run_bass_kernel_spmd

F32 = mybir.dt.float32
F32R = mybir.dt.float32r
F16 = mybir.dt.float16
F8 = mybir.dt.float8e4
DR = mybir.MatmulPerfMode.DoubleRow

B, N, C = 4, 4096, 512
HALF = N // 2          # own query rows per core
G = 32                 # groups
GS = C // G            # channels per group
P = 128                # partitions
CO = C // P            # channel subtiles (4)
CPAIR = CO // 2        # channel subtile pairs for DoubleRow (2)
N_CORES = 8
EPS = 1e-6
SM_SCALE = 1.0 / float(np.sqrt(C))
SHIFT = 3.0            # exp(score - SHIFT); max score ~6.7 -> exp <= 42
WSCALE = 16.0          # host scales w by 16 so s*w lands in fp8 normals
PIPELINE = False        # pre-emit next chunk's scores before the tail
I_CHUNK = 512          # query-chunk per attention sweep
N_CHUNKS = HALF // I_CHUNK   # 4
JT = N // P            # 32 key tiles
JPAIR = JT // 2        # 16 key-tile pairs (DoubleRow contracts 256 keys)
NT = N // P            # 32 row tiles per batch
AF = mybir.ActivationFunctionType
ALU = mybir.AluOpType


def _f(ap):
    return ap.bitcast(F32)


def build_nc():
    nc = bacc.Bacc("TRN2", target_bir_lowering=False, num_devices=N_CORES)

    xT8_d = nc.dram_tensor("xT8", [C, N], F8, kind="ExternalInput")
    wq_d = nc.dram_tensor("wq16", [C, C], F16, kind="ExternalInput")
    wk_d = nc.dram_tensor("wk16", [C, C], F16, kind="ExternalInput")
    wv_d = nc.dram_tensor("wv16", [C, C], F16, kind="ExternalInput")
    wo_d = nc.dram_tensor("wo16", [C, C], F16, kind="ExternalInput")
    bq_d = nc.dram_tensor("bq", [C], F32R, kind="ExternalInput")
    bk_d = nc.dram_tensor("bk", [C], F32R, kind="ExternalInput")
    bv_d = nc.dram_tensor("bv", [C], F32R, kind="ExternalInput")
    gamma_d = nc.dram_tensor("gn_gamma", [C], F32R, kind="ExternalInput")
    beta_d = nc.dram_tensor("gn_beta", [C], F32R, kind="ExternalInput")
    consts_d = nc.dram_tensor("consts", [P, 900], F32R, kind="ExternalInput")
    xbo_d = nc.dram_tensor("xbo", [HALF, C], F32R, kind="ExternalInput")
    out_d = nc.dram_tensor("out", [HALF, C], F32, kind="ExternalOutput")

    xbo_t = xbo_d[:].rearrange("(t p) c -> t p c", p=P)   # 16 x [128, 512]
    out_t = out_d[:].rearrange("(t p) c -> t p c", p=P)   # 16 x [128, 512]

    with tile.TileContext(nc) as tc:
        with (
            tc.tile_pool(name="persist", bufs=1) as persist,
            tc.tile_pool(name="cpool", bufs=1) as cpool,
            tc.tile_pool(name="keep", bufs=1) as keep,
        ):
            kT8 = persist.tile([P, CO, N], F8, tag="kT8")
            qT8 = persist.tile([P, CO, HALF], F8, tag="qT8")
            v8 = persist.tile([P, NT, C], F8, tag="v8")

            consts = cpool.tile([P, 900], F32R, tag="consts")
            nc.sync.dma_start(consts[:], consts_d[:])
            ident = consts[:, 0:P]
            ones_col = consts[:, P:P + 1]
            ones_row = consts[0:1, 129:257]
            nshift_col = consts[:, 257:258]   # all -SHIFT
            eps_col = consts[:, 258:259]      # all EPS

            parts = keep.tile([P, 4 * CO], F32R, tag="parts")
            s_part = parts[:, 0:CO]
            t_part = parts[:, CO:2 * CO]
            bqp = parts[:, 2 * CO:3 * CO]
            bkp = parts[:, 3 * CO:4 * CO]
            tp16 = keep.tile([P, CO], F16, tag="tp16")
            bv_eff = keep.tile([1, C], F32R, tag="bv_eff")

            with (
                tc.tile_pool(name="xpool", bufs=1) as xpool,
                tc.tile_pool(name="w16p", bufs=1) as w16p,
                tc.tile_pool(name="w8p", bufs=1) as w8p,
                tc.tile_pool(name="sqp", bufs=2) as sqp,
                tc.tile_pool(name="spool", bufs=1) as spool,
                tc.tile_pool(name="prows", bufs=1) as prows,
                tc.tile_pool(name="stats_ps", bufs=1, space="PSUM") as stats_ps,
            ):
                # ---- input DMAs ----
                xT8 = xpool.tile([P, CO, N], F8, tag="xT8", name="xT8")
                for o in range(CO):
                    eng = nc.sync if o % 2 == 0 else nc.gpsimd
                    eng.dma_start(xT8[:, o, :], xT8_d[o * P:(o + 1) * P, :])

                w16 = {}
                for name, src_d in (("wq", wq_d), ("wk", wk_d), ("wv", wv_d)):
                    w = w16p.tile([P, CO, C], F16, tag=name, name=name)
                    for o in range(CO):
                        nc.sync.dma_start(w[:, o, :], src_d[o * P:(o + 1) * P, :])
                    w16[name] = w

                irows = prows.tile([1, 5 * C], F32R, tag="irows")
                gamma_row = irows[:, 0 * C:1 * C]
                beta_row = irows[:, 1 * C:2 * C]
                bq_row = irows[:, 2 * C:3 * C]
                bk_row = irows[:, 3 * C:4 * C]
                bv_row = irows[:, 4 * C:5 * C]
                for i, src_d in enumerate((gamma_d, beta_d, bq_d, bk_d, bv_d)):
                    nc.sync.dma_start(irows[:, i * C:(i + 1) * C],
                                      src_d[:][None, :])

                wrows = prows.tile([1, 4 * C], F32, tag="wrows")
                sum_row = wrows[:, 0 * C:1 * C]
                sq_row = wrows[:, 1 * C:2 * C]
                s_row = wrows[:, 2 * C:3 * C].bitcast(F32R)
                t_row = wrows[:, 3 * C:4 * C].bitcast(F32R)
                berows = prows.tile([1, 2 * C], F32R, tag="berows")
                grows = prows.tile([1, 3 * G], F32, tag="grows")
                g_mean = grows[:, 0:G]
                g_var = grows[:, G:2 * G]
                g_tmp = grows[:, 2 * G:3 * G]

                # ---- group-norm stats from fp8 xT ----
                # statblk cols 0:4 = channel sums (per o), 4:8 = sumsq
                statblk = spool.tile([P, 8], F32, tag="statblk")
                for o in range(CO):
                    nc.vector.reduce_sum(statblk[:, o:o + 1], xT8[:, o, :],
                                         axis=mybir.AxisListType.X)
                # sum of squares on ScalarE (Square + accum_out; the DVE
                # tensor_tensor_reduce accum path crashes on hardware)
                for o in range(CO):
                    sqd = sqp.tile([P, N], F8, tag="sqd", name="sqd")
                    nc.scalar.activation(sqd[:], xT8[:, o, :], AF.Square,
                                         accum_out=statblk[:, 4 + o:5 + o])

                # transpose each [128,1] stat column into row layout
                with tc.tile_pool(name="stps", bufs=1,
                                  space="PSUM") as stps_pool:
                    sums_ps = stps_pool.tile([1, C], F32, tag="sums",
                                             name="sums_ps")
                    sqs_ps = stps_pool.tile([1, C], F32, tag="sqs",
                                            name="sqs_ps")
                    for o in range(CO):
                        nc.tensor.matmul(sums_ps[0:1, o * P:(o + 1) * P],
                                         statblk[:, o:o + 1], _f(ident),
                                         is_transpose=True,
                                         start=(o == 0), stop=(o == CO - 1))
                    for o in range(CO):
                        nc.tensor.matmul(sqs_ps[0:1, o * P:(o + 1) * P],
                                         statblk[:, 4 + o:5 + o], _f(ident),
                                         is_transpose=True,
                                         start=(o == 0), stop=(o == CO - 1))
                    nc.vector.tensor_copy(sum_row, sums_ps[:])
                    nc.vector.tensor_copy(sq_row, sqs_ps[:])

                # ---- group stats -> per-channel scale/shift ----
                # fused reduce: [sum_row | sq_row] -> [g_mean | g_var]
                inv_cnt = 1.0 / (N * GS)
                nc.vector.reduce_sum(
                    grows[:, 0:2 * G],
                    wrows[:, 0:2 * C].rearrange("p (g e) -> p g e", e=GS),
                    axis=mybir.AxisListType.X)
                nc.vector.tensor_scalar_mul(grows[:, 0:2 * G],
                                            grows[:, 0:2 * G], inv_cnt)
                nc.vector.tensor_mul(g_tmp, g_mean, g_mean)
                nc.vector.tensor_sub(g_var, g_var, g_tmp)
                nc.scalar.activation(g_tmp, g_var, AF.Sqrt,
                                     bias=_f(eps_col[0:1, :]))
                nc.vector.reciprocal(g_tmp, g_tmp)  # rstd per group

                sv = s_row.rearrange("p (g e) -> p g e", e=GS)
                tv = t_row.rearrange("p (g e) -> p g e", e=GS)
                gv = gamma_row.rearrange("p (g e) -> p g e", e=GS)
                nc.vector.tensor_tensor(
                    sv, gv, g_tmp[:, :, None].to_broadcast((1, G, GS)),
                    ALU.mult)
                nc.vector.tensor_tensor(
                    tv, sv, g_mean[:, :, None].to_broadcast((1, G, GS)),
                    ALU.mult)
                nc.vector.tensor_sub(t_row, beta_row, t_row)

                # ---- projections (fp8 DoubleRow; psum holds 16x values) ----
                def stage_out(dst, ps, bias_part, idx):
                    """psum/16 + bias -> fp8, alternating ScalarE / DVE."""
                    if idx % 2 == 0:
                        nc.scalar.activation(dst, ps, AF.Identity,
                                             bias=_f(bias_part),
                                             scale=1.0 / WSCALE)
                    else:
                        nc.vector.scalar_tensor_tensor(
                            dst, ps, 1.0 / WSCALE,
                            _f(bias_part).to_broadcast((P, ps.shape[-1])),
                            ALU.mult, ALU.add)

                with (
                    tc.tile_pool(name="proj_ps", bufs=1,
                                 space="PSUM") as proj_ps,
                    tc.tile_pool(name="pize_ps", bufs=1,
                                 space="PSUM") as pize_ps,
                ):
                    # s/t -> partition-broadcast form, single psum group
                    pp = pize_ps.tile([P, 2 * CO], F32, tag="pize", name="pp")
                    for i, vec_row in enumerate((s_row, t_row)):
                        for o in range(CO):
                            nc.tensor.matmul(
                                pp[:, i * CO + o:i * CO + o + 1],
                                _f(vec_row[0:1, o * P:(o + 1) * P]),
                                _f(ones_row[0:1, 0:1]),
                                start=(i == 0 and o == 0),
                                stop=(i == 1 and o == CO - 1))
                    nc.vector.tensor_copy(parts[:, 0:2 * CO], pp[:])
                    nc.vector.tensor_copy(tp16[:], pp[:, CO:2 * CO])

                    beff = {"wq": berows[:, 0:C], "wk": berows[:, C:2 * C],
                            "wv": bv_eff[:]}
                    brows = {"wq": bq_row, "wk": bk_row, "wv": bv_row}
                    w8 = {}

                    def fold_weight(name):
                        """Fold the group-norm scale into an fp8 weight copy
                        (only needs s_part; overlaps the bias matmuls)."""
                        w = w8p.tile([P, CO, C], F8, tag=name,
                                     name=f"{name}8")
                        for o in range(CO):
                            if o % 2 == 0:
                                nc.vector.tensor_scalar_mul(
                                    w[:, o, :], w16[name][:, o, :],
                                    _f(s_part[:, o:o + 1]))
                            else:
                                nc.scalar.activation(
                                    w[:, o, :], w16[name][:, o, :], AF.Copy,
                                    scale=_f(s_part[:, o:o + 1]))
                        w8[name] = w

                    def emit_beff(name):
                        """beff = t@w/16 + b (PE matmuls + DVE fixup)."""
                        bps = stats_ps.tile([1, C], F32, tag=f"bps{name}",
                                            name="bps")
                        for o in range(CO):
                            nc.tensor.matmul(bps[:], tp16[:, o:o + 1],
                                             w16[name][:, o, :],
                                             start=(o == 0),
                                             stop=(o == CO - 1))
                        nc.vector.scalar_tensor_tensor(
                            beff[name], bps[:], 1.0 / WSCALE, brows[name],
                            ALU.mult, ALU.add)

                    def emit_pize(name, bias_dst):
                        """beff row -> per-partition bias block."""
                        pb = pize_ps.tile([P, 2 * CO], F32, tag="pize",
                                          name="pb")
                        for o in range(CO):
                            nc.tensor.matmul(
                                pb[:, o:o + 1],
                                _f(beff[name][0:1, o * P:(o + 1) * P]),
                                _f(ones_row[0:1, 0:1]),
                                start=(o == 0), stop=(o == CO - 1))
                        nc.vector.tensor_copy(bias_dst, pb[:, 0:CO])

                    fold_weight("wk")
                    fold_weight("wq")
                    fold_weight("wv")
                    emit_beff("wk")
                    emit_pize("wk", bkp)
                    # wq/wv bias matmuls go before K-proj; their pize waits
                    # on the DVE fixup, so it is deferred past K-proj's 24us
                    # of matmuls where the dependency resolves with slack
                    emit_beff("wq")
                    emit_beff("wv")
                    # K: all 4096 keys
                    for o in range(CO):
                        for jcb in range(2):
                            kpss = [proj_ps.tile([P, 512], F32,
                                                 tag=f"proj{jc}",
                                                 name=f"kps{jc}")
                                    for jc in range(4)]
                            for cp in range(CPAIR):
                                for jc in range(4):
                                    col = (jcb * 4 + jc) * 512
                                    nc.tensor.matmul(
                                        kpss[jc][:],
                                        w8["wk"][:, 2 * cp:2 * cp + 2,
                                                 o * P:(o + 1) * P],
                                        xT8[:, 2 * cp:2 * cp + 2,
                                            col:col + 512],
                                        start=(cp == 0),
                                        stop=(cp == CPAIR - 1),
                                        perf_mode=DR)
                            for jc in range(4):
                                col = (jcb * 4 + jc) * 512
                                stage_out(kT8[:, o, col:col + 512],
                                          kpss[jc][:], bkp[:, o:o + 1], jc)

                    emit_pize("wq", bqp)
                    # Q: own 2048 queries
                    for o in range(CO):
                        qpss = [proj_ps.tile([P, 512], F32, tag=f"proj{jc}",
                                             name=f"qps{jc}")
                                for jc in range(4)]
                        for cp in range(CPAIR):
                            for jc in range(4):
                                nc.tensor.matmul(
                                    qpss[jc][:],
                                    w8["wq"][:, 2 * cp:2 * cp + 2,
                                             o * P:(o + 1) * P],
                                    xT8[:, 2 * cp:2 * cp + 2,
                                        jc * 512:(jc + 1) * 512],
                                    start=(cp == 0), stop=(cp == CPAIR - 1),
                                    perf_mode=DR)
                        for jc in range(4):
                            stage_out(qT8[:, o, jc * 512:(jc + 1) * 512],
                                      qpss[jc][:], bqp[:, o:o + 1], jc + 1)


                    # V rows (bias folded in later via denom outer-product)
                    for t16 in range(NT):
                        vps = proj_ps.tile([P, C], F32, tag=f"proj{t16 % 4}",
                                           name="vps")
                        for cp in range(CPAIR):
                            nc.tensor.matmul(
                                vps[:],
                                xT8[:, 2 * cp:2 * cp + 2,
                                    t16 * P:(t16 + 1) * P],
                                w8["wv"][:, 2 * cp:2 * cp + 2, :],
                                start=(cp == 0), stop=(cp == CPAIR - 1),
                                perf_mode=DR)
                        if t16 % 2 == 0:
                            nc.vector.tensor_scalar_mul(v8[:, t16, :], vps[:],
                                                        1.0 / WSCALE)
                        else:
                            nc.scalar.activation(v8[:, t16, :], vps[:],
                                                 AF.Copy, scale=1.0 / WSCALE)

            # ---- attention + output projection + residual ----
            with (
                tc.tile_pool(name="wop", bufs=1) as wop,
                tc.tile_pool(name="sT_ps", bufs=2, space="PSUM") as sT_ps,
                tc.tile_pool(name="av_ps", bufs=1, space="PSUM") as av_ps,
                tc.tile_pool(name="sh_ps", bufs=2, space="PSUM") as sh_ps,
                tc.tile_pool(name="expp", bufs=6 if PIPELINE else 4) as expp,
                tc.tile_pool(name="accp", bufs=2) as accp,
                tc.tile_pool(name="aoT", bufs=2) as aoTp,
                tc.tile_pool(name="ostage", bufs=2) as ostage,
                tc.tile_pool(name="xres", bufs=2) as xres,
                tc.tile_pool(name="drow", bufs=2) as drow,
            ):
                wo16 = wop.tile([P, CO, C], F16, tag="wo", name="wo16")
                for o in range(CO):
                    nc.sync.dma_start(wo16[:, o, :], wo_d[o * P:(o + 1) * P, :])

                PRE = 2   # score-pairs of chunk c+1 emitted before c's tail

                def emit_scores(chunk, jp):
                    """scores + exp for key-tile pair jp; returns ex tile."""
                    i0 = chunk * I_CHUNK
                    ex = expp.tile([P, 2, I_CHUNK], F8, tag="ex")
                    for jj in range(2):
                        j = 2 * jp + jj
                        sps = sT_ps.tile([P, I_CHUNK], F32, tag="sT",
                                         name="sps")
                        for cp in range(CPAIR):
                            nc.tensor.matmul(
                                sps[:],
                                kT8[:, 2 * cp:2 * cp + 2, j * P:(j + 1) * P],
                                qT8[:, 2 * cp:2 * cp + 2, i0:i0 + I_CHUNK],
                                start=(cp == 0), stop=(cp == CPAIR - 1),
                                perf_mode=DR)
                        nc.scalar.activation(ex[:, jj, :], sps[:], AF.Exp,
                                             scale=SM_SCALE,
                                             bias=_f(nshift_col))
                    return ex

                def emit_denom(accs, jp, ex):
                    """denominator partials on DVE (GpSimd fp8 ucode work
                    slows every other engine; keep it off the hot loop)."""
                    acc_a, acc_b = accs
                    for jj in range(2):
                        j = 2 * jp + jj
                        if j == 0:
                            nc.vector.tensor_copy(acc_a[:], ex[:, jj, :])
                        elif j == 1:
                            nc.vector.tensor_copy(acc_b[:], ex[:, jj, :])
                        elif j % 2 == 0:
                            nc.vector.tensor_add(acc_a[:], acc_a[:],
                                                 ex[:, jj, :])
                        else:
                            nc.vector.tensor_add(acc_b[:], acc_b[:],
                                                 ex[:, jj, :])

                def emit_av(avs, jp, ex):
                    for cs in range(CO):
                        nc.tensor.matmul(avs[cs][:],
                                         v8[:, 2 * jp:2 * jp + 2,
                                            cs * P:(cs + 1) * P],
                                         ex[:, :, :],
                                         start=(jp == 0), stop=False,
                                         perf_mode=DR)

                pending = []   # pre-emitted (jp, ex) for the next chunk
                for chunk in range(N_CHUNKS):
                    avs = [av_ps.tile([P, I_CHUNK], F32, tag=f"av{i}",
                                      name=f"av{i}")
                           for i in range(CO)]
                    acc_a = accp.tile([P, I_CHUNK], F32, tag="acc_a",
                                      name="acc_a")
                    acc_b = accp.tile([P, I_CHUNK], F32, tag="acc_b",
                                      name="acc_b")
                    accs = (acc_a, acc_b)
                    for jp, ex in pending:
                        emit_denom(accs, jp, ex)
                        emit_av(avs, jp, ex)
                    # one-pair lookahead: scores of pair p+1 are emitted
                    # before the AV matmuls of pair p, so the PE does not
                    # idle on pair p's exp (ScalarE) latency
                    prev = None
                    for jp in range(len(pending), JPAIR):
                        ex = emit_scores(chunk, jp)
                        emit_denom(accs, jp, ex)
                        if prev is not None:
                            emit_av(avs, *prev)
                        prev = (jp, ex)
                    if prev is not None:
                        emit_av(avs, *prev)

                    # pre-emit next chunk's first score pairs so the PE has
                    # work queued while this chunk's tail waits on the
                    # denominator chain
                    pending = []
                    if PIPELINE and chunk + 1 < N_CHUNKS:
                        for jp in range(PRE):
                            pending.append((jp, emit_scores(chunk + 1, jp)))

                    # ---- chunk tail ----
                    dps = sh_ps.tile([1, I_CHUNK], F32, tag="sh", name="dps")
                    if PIPELINE:
                        nc.tensor.matmul(dps[:], _f(ones_col), _f(accs[0][:]),
                                         start=True, stop=False)
                        nc.tensor.matmul(dps[:], _f(ones_col), _f(accs[1][:]),
                                         start=False, stop=True)
                    else:
                        nc.vector.tensor_add(accs[0][:], accs[0][:],
                                             accs[1][:])
                        nc.tensor.matmul(dps[:], _f(ones_col), _f(accs[0][:]),
                                         start=True, stop=True)
                    d_row = drow.tile([1, I_CHUNK], F32R, tag="d_row")
                    nc.vector.tensor_copy(d_row[:], dps[:])
                    # V-bias: avT += bv (x) denom (unnormalized rows sum to denom)
                    for cs in range(CO):
                        nc.tensor.matmul(avs[cs][:],
                                         bv_eff[0:1, cs * P:(cs + 1) * P],
                                         d_row[:],
                                         start=False, stop=True)
                    dp = sh_ps.tile([P, 4], F32, tag="sh", name="dp")
                    for o in range(4):
                        nc.tensor.matmul(dp[:, o:o + 1],
                                         _f(d_row[0:1, o * P:(o + 1) * P]),
                                         _f(ones_row[0:1, 0:1]),
                                         start=(o == 0), stop=(o == 3))
                    d_inv = drow.tile([P, 4], F32, tag="d_inv")
                    nc.vector.reciprocal(d_inv[:], dp[:])

                    aoT = aoTp.tile([P, CO, I_CHUNK], F16, tag="aoT")
                    for cs in range(CO):
                        if cs % 2 == 0:
                            nc.vector.tensor_copy(aoT[:, cs, :], avs[cs][:])
                        else:
                            nc.scalar.activation(aoT[:, cs, :], avs[cs][:],
                                                 AF.Copy)

                    for it in range(4):
                        ops = sh_ps.tile([P, C], F32, tag="sh", name="ops")
                        for ci in range(CO):
                            nc.tensor.matmul(ops[:],
                                             aoT[:, ci, it * P:(it + 1) * P],
                                             wo16[:, ci, :],
                                             start=(ci == 0),
                                             stop=(ci == CO - 1))
                        xr = xres.tile([P, C], F32R, tag="xr")
                        nc.sync.dma_start(xr[:], xbo_t[chunk * 4 + it])
                        ot = ostage.tile([P, C], F32, tag="ot")
                        nc.vector.scalar_tensor_tensor(
                            ot[:], ops[:], _f(d_inv[:, it:it + 1]), xr[:],
                            ALU.mult, ALU.add)
                        nc.sync.dma_start(out_t[chunk * 4 + it], ot[:])

    nc.compile()
    return nc


_NC = None


def _get_nc():
    global _NC
    if _NC is None:
        _NC = build_nc()
    return _NC


def make_consts():
    consts = np.zeros((P, 900), np.float32)
    consts[:, 0:P] = np.eye(P, dtype=np.float32)
    consts[:, P] = 1.0
    consts[0, 129:257] = 1.0
    consts[:, 257] = -SHIFT
    consts[:, 258] = EPS
    return consts


def make_in_maps(x, gn_gamma, gn_beta, wq, bq, wk, bk, wv, bv, wo, bo):
    x4 = np.ascontiguousarray(np.asarray(x, np.float32).reshape(B, N, C))
    consts = make_consts()
    bo_f = np.asarray(bo, np.float32)
    common = dict(
        wq16=(WSCALE * np.asarray(wq, np.float32)).astype(np.float16),
        wk16=(WSCALE * np.asarray(wk, np.float32)).astype(np.float16),
        wv16=(WSCALE * np.asarray(wv, np.float32)).astype(np.float16),
        wo16=np.asarray(wo, np.float32).astype(np.float16),
        bq=np.asarray(bq, np.float32), bk=np.asarray(bk, np.float32),
        bv=np.asarray(bv, np.float32),
        gn_gamma=np.asarray(gn_gamma, np.float32),
        gn_beta=np.asarray(gn_beta, np.float32),
        consts=consts,
    )
    in_maps = []
    for c in range(N_CORES):
        b, h = c // 2, c % 2
        own = x4[b, h * HALF:(h + 1) * HALF]
        other = x4[b, (1 - h) * HALF:(2 - h) * HALF]
        xb_ = np.concatenate([own, other], axis=0)          # [N, C]
        xT8 = np.ascontiguousarray(xb_.T).astype(ml_dtypes.float8_e4m3)
        xbo = np.ascontiguousarray(own + bo_f)
        in_maps.append(dict(xT8=xT8, xbo=xbo, **common))
    return in_maps


def assemble(results):
    out = np.empty((B, N, C), np.float32)
    for c in range(N_CORES):
        b, h = c // 2, c % 2
        out[b, h * HALF:(h + 1) * HALF] = results[c]["out"]
    return out.reshape(B, 64, 64, C)


def kernel(**inputs):
    nc = _get_nc()
    in_maps = make_in_maps(**inputs)
    res = run_bass_kernel_spmd(nc, in_maps, list(range(N_CORES)))
    return assemble(res.results)
